# revision 1
# baseline (speedup 1.0000x reference)
"""Neural CDE forward pass on 8 Trainium2 NeuronCores (pure data parallel).

B=512 batch is sharded 64 per core.  Per core, the entire 30-step RK4
integration (120 vector-field evals) runs out of SBUF:

  - small MLP layers feature-major: lhsT = weight chunk (stationary),
    rhs = activation [feat, 64]; bias+relu fused in one DVE tensor_scalar.
  - W_out layer batch-major, split-h: y PSUM [128 = 2 h-halves x 64 batch,
    cols = (h_local, i_pad)] so tanh/mul/reduce use all 128 lanes.
    b_out enters via a ones-row appended to the stationary activation.
  - einsum g[b,h] = sum_i tanh(y)[b,h,i] * dX[b,i]: DVE multiply with a
    broadcast dX tile (fp16) + segmented tensor_reduce over i (41 padded
    to 42 so segments stay 4B-aligned).
  - dX for the 75 distinct (interval, s) points is computed on host and
    DMA'd once.
  - z is transposed back to feature-major each stage with two PE
    transposes through stacked identity matrices.
"""

import os
import numpy as np

B, T, IN, HID, HH, NCLS = 512, 16, 41, 64, 150, 4
N_SUB, NL = 2, 3
NCORES = 8
BL = B // NCORES            # 64 per-core batch
INP = IN + 1                # 42: i padded for even segments
NIV = T - 1                 # 15 intervals
NS = 5                      # distinct s values per interval
HSTEP = 1.0 / N_SUB         # 0.5
HALF_COLS = 32 * INP        # 1344 cols per h-half
# (h0, hcount) splits of the 32 h per half; each region's cols (hcount*42)
# must fit one 2KB PSUM bank (<=512 fp32) since a matmul cannot cross banks
REGIONS = [(0, 12), (12, 12), (24, 8)]
STEPS = int(os.environ.get("NCDE_STEPS", NIV * N_SUB))  # debug knob


def _prep_shared(W0, b0, W_in, b_in, W_h, b_h, W_out, b_out, Wc1, bc1, Wc2, bc2):
    f16 = np.float16
    f32 = np.float32
    wha = np.concatenate([W_h[i][0:128, :] for i in range(NL)], axis=1)
    whb = np.concatenate([W_h[i][128:HH, :] for i in range(NL)], axis=1)
    bias_a = np.stack([b_in[0:128]] + [b_h[i][0:128] for i in range(NL)], axis=1)
    bias_b = np.stack([b_in[128:HH]] + [b_h[i][128:HH] for i in range(NL)], axis=1)
    R = W_out.reshape(HH, HID, IN)
    Rp = np.zeros((HH, HID, INP), np.float32)
    Rp[:, :, :IN] = R
    W2 = np.concatenate(
        [Rp[:, 0:32, :].reshape(HH, HALF_COLS), Rp[:, 32:64, :].reshape(HH, HALF_COLS)],
        axis=1,
    )
    bo = np.zeros((HID, INP), np.float32)
    bo[:, :IN] = b_out.reshape(HID, IN)
    bo2 = np.concatenate([bo[0:32].reshape(-1), bo[32:64].reshape(-1)])
    i64 = np.eye(64, dtype=np.float32)
    i32 = np.eye(32, dtype=np.float32)
    return {
        "w0": W0.astype(f16),
        "b0c": b0.reshape(HID, 1).astype(f32),
        "wi": W_in.astype(f16),
        "wha": wha.astype(f16),
        "whb": whb.astype(f16),
        "bias_a": bias_a.astype(f32),
        "bias_b": bias_b.astype(f32),
        "woa": W2[0:128].astype(f16),
        "wob": np.vstack([W2[128:HH], bo2[None]]).astype(f16),
        "wc1": Wc1.astype(f16),
        "bc1c": bc1.reshape(HID, 1).astype(f32),
        "wc2": Wc2.astype(f16),
        "bc2c": bc2.reshape(NCLS, 1).astype(f32),
        "eperm": np.vstack(
            [np.hstack([i64, 0 * i64]), np.hstack([0 * i64, i64])]
        ).astype(f16),
        "id64f": i64.astype(f32),
        "hhB_init": np.vstack(
            [np.zeros((HH - 128, BL), np.float32), np.ones((1, BL), np.float32)]
        ).astype(f16),
    }


def _prep_percore(bc_core):
    """bc_core: [BL, NIV, 4, IN] fp32 -> x0t [IN, BL] f16, dxh [128, NS*NIV*INP] f16."""
    x0t = bc_core[:, 0, 0, :].T.astype(np.float16)
    c1 = bc_core[:, :, 1, :]  # [BL, NIV, IN]
    c2 = bc_core[:, :, 2, :]
    c3 = bc_core[:, :, 3, :]
    dxh = np.zeros((128, NIV * NS * INP), np.float32)
    for iv in range(NIV):
        for si in range(NS):
            s = si * 0.25
            dX = c1[:, iv] + (2.0 * s) * c2[:, iv] + (3.0 * s * s) * c3[:, iv]
            col = (iv * NS + si) * INP
            dxh[0:BL, col : col + IN] = dX
            dxh[BL:128, col : col + IN] = dX
    return x0t, dxh.astype(np.float16)


def build_nc(steps=STEPS):
    """Build the single-core Bass program (same program on all 8 cores)."""
    from contextlib import ExitStack

    import concourse.bass as bass
    import concourse.mybir as mybir
    from concourse import bacc, tile

    f16 = mybir.dt.float16
    f32 = mybir.dt.float32
    AF = mybir.ActivationFunctionType
    OP = mybir.AluOpType

    nc = bacc.Bacc("TRN2", target_bir_lowering=False, debug=False)

    dram = {}
    ins_spec = [
        ("x0t", [IN, BL], f16),
        ("dxh", [128, NIV * NS * INP], f16),
        ("w0", [IN, HID], f16),
        ("b0c", [HID, 1], f32),
        ("wi", [HID, HH], f16),
        ("wha", [128, NL * HH], f16),
        ("whb", [HH - 128, NL * HH], f16),
        ("bias_a", [128, 1 + NL], f32),
        ("bias_b", [HH - 128, 1 + NL], f32),
        ("woa", [128, 2 * HALF_COLS], f16),
        ("wob", [HH - 128 + 1, 2 * HALF_COLS], f16),
        ("wc1", [HID, HID], f16),
        ("bc1c", [HID, 1], f32),
        ("wc2", [HID, NCLS], f16),
        ("bc2c", [NCLS, 1], f32),
        ("eperm", [128, 128], f16),
        ("id64f", [64, 64], f32),
        ("hhB_init", [HH - 128 + 1, BL], f16),
    ]
    for name, shape, dt in ins_spec:
        dram[name] = nc.dram_tensor(name, shape, dt, kind="ExternalInput")
    out_dram = nc.dram_tensor("pred_t", [NCLS, BL], f32, kind="ExternalOutput")

    with tile.TileContext(nc) as tc:
        with ExitStack() as ctx:
            const = ctx.enter_context(tc.tile_pool(name="const", bufs=1))
            work = ctx.enter_context(tc.tile_pool(name="work", bufs=3))
            ty_pool = ctx.enter_context(tc.tile_pool(name="ty", bufs=3))
            pr_pool = ctx.enter_context(tc.tile_pool(name="pr", bufs=3))
            ps_h = ctx.enter_context(
                tc.tile_pool(name="ps_h", bufs=2, space=bass.MemorySpace.PSUM)
            )
            ps_hb = ctx.enter_context(
                tc.tile_pool(name="ps_hb", bufs=1, space=bass.MemorySpace.PSUM)
            )
            ps_y = ctx.enter_context(
                tc.tile_pool(name="ps_y", bufs=1, space=bass.MemorySpace.PSUM)
            )
            ps_t = ctx.enter_context(
                tc.tile_pool(name="ps_t", bufs=2, space=bass.MemorySpace.PSUM)
            )

            # ---- load constants/weights into SBUF --------------------------
            sb = {}
            for name, shape, dt in ins_spec:
                t = const.tile(shape, dt, tag=name)
                nc.sync.dma_start(t[:], dram[name][:])
                sb[name] = t

            # persistent state tiles (hhB arrives with its ones row preset)
            hhB = sb["hhB_init"]
            zA = const.tile([128, 32], f32, tag="zA")
            zB = const.tile([128, 32], f32, tag="zB")

            # psum y region tiles (persistent; serial stages reuse them)
            # one full 2KB bank each so every tile starts bank-aligned
            yR = [
                ps_y.tile([128, 512], f32, tag=f"yR{rt}", name=f"yR{rt}")
                for rt in range(len(REGIONS))
            ]

            # ---- initial state z0 = X0 @ W0 + b0 ---------------------------
            z0p = ps_h.tile([HID, BL], f32, tag="hA")
            nc.tensor.matmul(z0p[:], sb["w0"][:], sb["x0t"][:])
            zT = work.tile([HID, BL], f16, tag="zT")
            nc.vector.tensor_scalar(zT[:], z0p[:], sb["b0c"][:], None, OP.add)
            z0s = work.tile([HID, BL], f32, tag="z0s")
            nc.vector.tensor_scalar(z0s[:], z0p[:], sb["b0c"][:], None, OP.add)
            # permutation matmuls: z0s (feature-major) -> z (split-h bm fp32)
            ztp0 = ps_t.tile([128, 32], f32, tag="ztp")
            nc.tensor.matmul(ztp0[0:64, :], z0s[:], sb["id64f"][:, 0:32])
            nc.tensor.matmul(ztp0[64:128, :], z0s[:], sb["id64f"][:, 32:64])
            z = zA
            zacc = zB
            nc.vector.tensor_copy(z[:], ztp0[:])

            # RK4 coefficients
            acc_w = [HSTEP / 6.0, HSTEP / 3.0, HSTEP / 3.0, HSTEP / 6.0]
            inp_w = [0.5 * HSTEP, 0.5 * HSTEP, HSTEP, None]

            # ---- time stepping --------------------------------------------
            for step in range(steps):
                iv, sub = step // N_SUB, step % N_SUB
                for stg in range(4):
                    sidx = 2 * sub + (0 if stg == 0 else (1 if stg < 3 else 2))
                    dxcol = (iv * NS + sidx) * INP

                    # -- small MLP: W_in then NL hidden layers (feature-major)
                    hA = None
                    hB = None
                    for layer in range(1 + NL):
                        if layer == 0:
                            wa_l = sb["wi"][:]
                            wb_l = None
                        else:
                            c0 = (layer - 1) * HH
                            wa_l = sb["wha"][:, c0 : c0 + HH]
                            wb_l = sb["whb"][:, c0 : c0 + HH]
                        pA = ps_h.tile([128, BL], f32, tag="hA")
                        pB = ps_hb.tile([HH - 128, BL], f32, tag="hB")
                        if layer == 0:
                            nc.tensor.matmul(pA[:], wa_l[:, 0:128], zT[:])
                            nc.tensor.matmul(pB[:], wa_l[:, 128:HH], zT[:])
                        else:
                            nc.tensor.matmul(
                                pA[:], wa_l[:, 0:128], hA[:], start=True, stop=False
                            )
                            nc.tensor.matmul(
                                pA[:], wb_l[:, 0:128], hB[:], start=False, stop=True
                            )
                            nc.tensor.matmul(
                                pB[:], wa_l[:, 128:HH], hA[:], start=True, stop=False
                            )
                            nc.tensor.matmul(
                                pB[:], wb_l[:, 128:HH], hB[:], start=False, stop=True
                            )
                        last = layer == NL
                        nhA = work.tile([128, BL], f16, tag="hA_sb")
                        nhB = hhB[0 : HH - 128, :] if last else work.tile(
                            [HH - 128, BL], f16, tag="hB_sb"
                        )
                        ba = sb["bias_a"][:, layer : layer + 1]
                        bb = sb["bias_b"][:, layer : layer + 1]
                        # A on ACT, B on DVE: the two bias+relu ops run
                        # concurrently instead of queueing on one engine
                        nc.scalar.activation(nhA[:], pA[:], AF.Relu, bias=ba)
                        nc.vector.tensor_scalar(nhB[:], pB[:], bb, 0.0, OP.add, OP.max)
                        hA, hB = nhA, (hhB[0 : HH - 128 + 1, :] if last else nhB)

                    # -- W_out: y[p = half*64+b, (h_local, i)]  (batch-major)
                    # region-major so each region completes early and the
                    # tanh/einsum chain starts while later regions stream
                    for rt, (h0, hc) in enumerate(REGIONS):
                        for kc in range(2):
                            lhs = hA[:] if kc == 0 else hhB[:]
                            rhs_t = sb["woa"] if kc == 0 else sb["wob"]
                            for half in range(2):
                                cols = half * HALF_COLS + h0 * INP
                                # lo/hi halves accumulate in disjoint
                                # partition rows of one bank; the sim's group
                                # guard is partition-blind, so skip it.
                                nc.tensor.matmul(
                                    yR[rt][half * 64 : half * 64 + 64, 0 : hc * INP],
                                    lhs,
                                    rhs_t[:, cols : cols + hc * INP],
                                    start=(kc == 0),
                                    stop=(kc == 1),
                                    skip_group_check=True,
                                )

                    # -- tanh -> multiply by dX -> segmented reduce over i
                    k_t = work.tile([128, 32], f32, tag="k")
                    for rt, (h0, hc) in enumerate(REGIONS):
                        ty = ty_pool.tile([128, hc * INP], f16, tag=f"ty{rt}")
                        nc.scalar.activation(ty[:], yR[rt][:, 0 : hc * INP], AF.Tanh)
                        pr = pr_pool.tile([128, hc * INP], f16, tag=f"pr{rt}")
                        dxv = (
                            sb["dxh"][:, dxcol : dxcol + INP]
                            .unsqueeze(1)
                            .broadcast_to((128, hc, INP))
                        )
                        tyv = ty[:].rearrange("p (h i) -> p h i", i=INP)
                        prv = pr[:].rearrange("p (h i) -> p h i", i=INP)
                        nc.vector.tensor_tensor(prv, tyv, dxv, OP.mult)
                        nc.vector.tensor_reduce(
                            k_t[:, h0 : h0 + hc], prv, mybir.AxisListType.X, OP.add
                        )

                    # -- next-stage input first (it gates the transpose and
                    # the whole next stage), then the zacc accumulation
                    zs16 = work.tile([128, 32], f16, tag="zs16")
                    if stg < 3:
                        nc.vector.scalar_tensor_tensor(
                            zs16[:], k_t[:], inp_w[stg], z[:], OP.mult, OP.add
                        )
                    if stg == 0:
                        nc.vector.scalar_tensor_tensor(
                            zacc[:], k_t[:], acc_w[0], z[:], OP.mult, OP.add
                        )
                    else:
                        nc.vector.scalar_tensor_tensor(
                            zacc[:], k_t[:], acc_w[stg], zacc[:], OP.mult, OP.add
                        )
                    if stg == 3:
                        nc.vector.tensor_copy(zs16[:], zacc[:])
                    ztp = ps_t.tile([HID, BL], f32, tag="ztp")
                    nc.tensor.matmul(ztp[0:32, :], zs16[:], sb["eperm"][:, 0:64])
                    nc.tensor.matmul(ztp[32:64, :], zs16[:], sb["eperm"][:, 64:128])
                    zT = work.tile([HID, BL], f16, tag="zT")
                    nc.vector.tensor_copy(zT[:], ztp[:])
                    if stg == 3:
                        z, zacc = zacc, z

            # ---- classifier on final state --------------------------------
            c1p = ps_h.tile([HID, BL], f32, tag="hA")
            nc.tensor.matmul(c1p[:], sb["wc1"][:], zT[:])
            c1 = work.tile([HID, BL], f16, tag="c1")
            nc.vector.tensor_scalar(c1[:], c1p[:], sb["bc1c"][:], 0.0, OP.add, OP.max)
            c2p = ps_hb.tile([NCLS, BL], f32, tag="hB")
            nc.tensor.matmul(c2p[:], sb["wc2"][:], c1[:])
            pred = work.tile([NCLS, BL], f32, tag="pred")
            nc.vector.tensor_scalar(pred[:], c2p[:], sb["bc2c"][:], None, OP.add)
            nc.sync.dma_start(out_dram[:], pred[:])

    nc.compile()
    return nc


def make_in_maps(inputs):
    shared = _prep_shared(
        inputs["W0"], inputs["b0"], inputs["W_in"], inputs["b_in"],
        inputs["W_h"], inputs["b_h"], inputs["W_out"], inputs["b_out"],
        inputs["Wc1"], inputs["bc1"], inputs["Wc2"], inputs["bc2"],
    )
    bc = np.asarray(inputs["batch_coeffs"], np.float32)
    in_maps = []
    for c in range(NCORES):
        x0t, dxh = _prep_percore(bc[c * BL : (c + 1) * BL])
        in_maps.append({**shared, "x0t": x0t, "dxh": dxh})
    return in_maps


_CACHED = {}


def kernel(**inputs):
    from concourse.bass_utils import run_bass_kernel_spmd

    if "nc" not in _CACHED:
        _CACHED["nc"] = build_nc()
    nc = _CACHED["nc"]
    in_maps = make_in_maps(inputs)
    res = run_bass_kernel_spmd(
        nc, in_maps, core_ids=list(range(NCORES)),
        trace=bool(int(os.environ.get("NCDE_TRACE", "0"))),
    )
    _CACHED["last_result"] = res
    out = np.zeros((B, NCLS), np.float32)
    for c in range(NCORES):
        out[c * BL : (c + 1) * BL, :] = res.results[c]["pred_t"].T
    return out



# revision 8
# speedup vs baseline: 1.0052x; 1.0052x over previous
"""Neural CDE forward pass on 8 Trainium2 NeuronCores (pure data parallel).

B=512 batch is sharded 64 per core.  Per core, the entire 30-step RK4
integration (120 vector-field evals) runs out of SBUF:

  - small MLP layers feature-major: lhsT = weight chunk (stationary),
    rhs = activation [feat, 64]; bias+relu fused in one DVE tensor_scalar.
  - W_out layer batch-major, split-h: y PSUM [128 = 2 h-halves x 64 batch,
    cols = (h_local, i_pad)] so tanh/mul/reduce use all 128 lanes.
    b_out enters via a ones-row appended to the stationary activation.
  - einsum g[b,h] = sum_i tanh(y)[b,h,i] * dX[b,i]: DVE multiply with a
    broadcast dX tile (fp16) + segmented tensor_reduce over i (41 padded
    to 42 so segments stay 4B-aligned).
  - dX for the 75 distinct (interval, s) points is computed on host and
    DMA'd once.
  - z is transposed back to feature-major each stage with two PE
    transposes through stacked identity matrices.
"""

import os
import numpy as np

B, T, IN, HID, HH, NCLS = 512, 16, 41, 64, 150, 4
N_SUB, NL = 2, 3
NCORES = 8
BL = B // NCORES            # 64 per-core batch
INP = IN + 1                # 42: i padded for even segments
NIV = T - 1                 # 15 intervals
NS = 5                      # distinct s values per interval
HSTEP = 1.0 / N_SUB         # 0.5
HALF_COLS = 32 * INP        # 1344 cols per h-half
# (h0, hcount) splits of the 32 h per half; each region's cols (hcount*42)
# must fit one 2KB PSUM bank (<=512 fp32) since a matmul cannot cross banks
REGIONS = [(0, 12), (12, 12), (24, 8)]
STEPS = int(os.environ.get("NCDE_STEPS", NIV * N_SUB))  # debug knob
# PE p-state fillers: TRN2 drops the PE clock (2.4 -> 1.2/0.65 GHz) when the
# engine idles >100ns; dummy matmuls into a scratch PSUM bank bridge the
# relu/einsum gaps so real matmuls stay at full clock.
FILL_MLP = int(os.environ.get("NCDE_FILL_MLP", "4"))    # per MLP layer, 128col
FILL_TAIL = int(os.environ.get("NCDE_FILL_TAIL", "8"))  # after W_out, 512col
FILL_POST = int(os.environ.get("NCDE_FILL_POST", "2"))  # after transpose, 256col


def _prep_shared(W0, b0, W_in, b_in, W_h, b_h, W_out, b_out, Wc1, bc1, Wc2, bc2):
    f16 = np.float16
    f32 = np.float32
    wha = np.concatenate([W_h[i][0:128, :] for i in range(NL)], axis=1)
    whb = np.concatenate([W_h[i][128:HH, :] for i in range(NL)], axis=1)
    bias_a = np.stack([b_in[0:128]] + [b_h[i][0:128] for i in range(NL)], axis=1)
    bias_b = np.stack([b_in[128:HH]] + [b_h[i][128:HH] for i in range(NL)], axis=1)
    R = W_out.reshape(HH, HID, IN)
    Rp = np.zeros((HH, HID, INP), np.float32)
    Rp[:, :, :IN] = R
    W2 = np.concatenate(
        [Rp[:, 0:32, :].reshape(HH, HALF_COLS), Rp[:, 32:64, :].reshape(HH, HALF_COLS)],
        axis=1,
    )
    bo = np.zeros((HID, INP), np.float32)
    bo[:, :IN] = b_out.reshape(HID, IN)
    bo2 = np.concatenate([bo[0:32].reshape(-1), bo[32:64].reshape(-1)])
    i64 = np.eye(64, dtype=np.float32)
    i32 = np.eye(32, dtype=np.float32)
    return {
        "w0": W0.astype(f16),
        "b0c": b0.reshape(HID, 1).astype(f32),
        "wi": W_in.astype(f16),
        "wha": wha.astype(f16),
        "whb": whb.astype(f16),
        "bias_a": bias_a.astype(f32),
        "bias_b": bias_b.astype(f32),
        "woa": W2[0:128].astype(f16),
        "wob": np.vstack([W2[128:HH], bo2[None]]).astype(f16),
        "wc1": Wc1.astype(f16),
        "bc1c": bc1.reshape(HID, 1).astype(f32),
        "wc2": Wc2.astype(f16),
        "bc2c": bc2.reshape(NCLS, 1).astype(f32),
        "eperm": np.vstack(
            [np.hstack([i64, 0 * i64]), np.hstack([0 * i64, i64])]
        ).astype(f16),
        "id64f": i64.astype(f32),
        "hhB_init": np.vstack(
            [np.zeros((HH - 128, BL), np.float32), np.ones((1, BL), np.float32)]
        ).astype(f16),
    }


def _prep_percore(bc_core):
    """bc_core: [BL, NIV, 4, IN] fp32 -> x0t [IN, BL] f16, dxh [128, NS*NIV*INP] f16."""
    x0t = bc_core[:, 0, 0, :].T.astype(np.float16)
    c1 = bc_core[:, :, 1, :]  # [BL, NIV, IN]
    c2 = bc_core[:, :, 2, :]
    c3 = bc_core[:, :, 3, :]
    dxh = np.zeros((128, NIV * NS * INP), np.float32)
    for iv in range(NIV):
        for si in range(NS):
            s = si * 0.25
            dX = c1[:, iv] + (2.0 * s) * c2[:, iv] + (3.0 * s * s) * c3[:, iv]
            col = (iv * NS + si) * INP
            dxh[0:BL, col : col + IN] = dX
            dxh[BL:128, col : col + IN] = dX
    return x0t, dxh.astype(np.float16)


def build_nc(steps=STEPS):
    """Build the single-core Bass program (same program on all 8 cores)."""
    from contextlib import ExitStack

    import concourse.bass as bass
    import concourse.mybir as mybir
    from concourse import bacc, tile

    f16 = mybir.dt.float16
    f32 = mybir.dt.float32
    AF = mybir.ActivationFunctionType
    OP = mybir.AluOpType

    nc = bacc.Bacc("TRN2", target_bir_lowering=False, debug=False)

    dram = {}
    ins_spec = [
        ("x0t", [IN, BL], f16),
        ("dxh", [128, NIV * NS * INP], f16),
        ("w0", [IN, HID], f16),
        ("b0c", [HID, 1], f32),
        ("wi", [HID, HH], f16),
        ("wha", [128, NL * HH], f16),
        ("whb", [HH - 128, NL * HH], f16),
        ("bias_a", [128, 1 + NL], f32),
        ("bias_b", [HH - 128, 1 + NL], f32),
        ("woa", [128, 2 * HALF_COLS], f16),
        ("wob", [HH - 128 + 1, 2 * HALF_COLS], f16),
        ("wc1", [HID, HID], f16),
        ("bc1c", [HID, 1], f32),
        ("wc2", [HID, NCLS], f16),
        ("bc2c", [NCLS, 1], f32),
        ("eperm", [128, 128], f16),
        ("id64f", [64, 64], f32),
        ("hhB_init", [HH - 128 + 1, BL], f16),
    ]
    for name, shape, dt in ins_spec:
        dram[name] = nc.dram_tensor(name, shape, dt, kind="ExternalInput")
    out_dram = nc.dram_tensor("pred_t", [NCLS, BL], f32, kind="ExternalOutput")

    with tile.TileContext(nc) as tc:
        with ExitStack() as ctx:
            const = ctx.enter_context(tc.tile_pool(name="const", bufs=1))
            work = ctx.enter_context(tc.tile_pool(name="work", bufs=3))
            ty_pool = ctx.enter_context(tc.tile_pool(name="ty", bufs=3))
            pr_pool = ctx.enter_context(tc.tile_pool(name="pr", bufs=3))
            ps_h = ctx.enter_context(
                tc.tile_pool(name="ps_h", bufs=2, space=bass.MemorySpace.PSUM)
            )
            ps_hb = ctx.enter_context(
                tc.tile_pool(name="ps_hb", bufs=1, space=bass.MemorySpace.PSUM)
            )
            ps_y = ctx.enter_context(
                tc.tile_pool(name="ps_y", bufs=1, space=bass.MemorySpace.PSUM)
            )
            ps_t = ctx.enter_context(
                tc.tile_pool(name="ps_t", bufs=1, space=bass.MemorySpace.PSUM)
            )
            ps_f = ctx.enter_context(
                tc.tile_pool(name="ps_f", bufs=1, space=bass.MemorySpace.PSUM)
            )

            # ---- load constants/weights into SBUF --------------------------
            sb = {}
            for name, shape, dt in ins_spec:
                t = const.tile(shape, dt, tag=name)
                nc.sync.dma_start(t[:], dram[name][:])
                sb[name] = t

            # persistent state tiles (hhB arrives with its ones row preset)
            hhB = sb["hhB_init"]
            zA = const.tile([128, 32], f32, tag="zA")
            zB = const.tile([128, 32], f32, tag="zB")

            # psum y region tiles (persistent; serial stages reuse them)
            # one full 2KB bank each so every tile starts bank-aligned
            yR = [
                ps_y.tile([128, 512], f32, tag=f"yR{rt}", name=f"yR{rt}")
                for rt in range(len(REGIONS))
            ]

            # scratch PSUM bank for p-state filler matmuls (results unread)
            fillp = ps_f.tile([128, 512], f32, tag="fillp", name="fillp")

            def pe_fill(n, cols):
                for _ in range(n):
                    nc.tensor.matmul(
                        fillp[:, 0:cols],
                        sb["eperm"][:],
                        sb["woa"][:, 0:cols],
                        skip_group_check=True,
                    )

            # ---- initial state z0 = X0 @ W0 + b0 ---------------------------
            z0p = ps_h.tile([HID, BL], f32, tag="hA")
            nc.tensor.matmul(z0p[:], sb["w0"][:], sb["x0t"][:])
            zT = work.tile([HID, BL], f16, tag="zT")
            nc.vector.tensor_scalar(zT[:], z0p[:], sb["b0c"][:], None, OP.add)
            z0s = work.tile([HID, BL], f32, tag="z0s")
            nc.vector.tensor_scalar(z0s[:], z0p[:], sb["b0c"][:], None, OP.add)
            # permutation matmuls: z0s (feature-major) -> z (split-h bm fp32)
            ztp0 = ps_t.tile([128, 32], f32, tag="ztp")
            nc.tensor.matmul(ztp0[0:64, :], z0s[:], sb["id64f"][:, 0:32])
            nc.tensor.matmul(ztp0[64:128, :], z0s[:], sb["id64f"][:, 32:64])
            z = zA
            zacc = zB
            nc.vector.tensor_copy(z[:], ztp0[:])

            # RK4 coefficients
            acc_w = [HSTEP / 6.0, HSTEP / 3.0, HSTEP / 3.0, HSTEP / 6.0]
            inp_w = [0.5 * HSTEP, 0.5 * HSTEP, HSTEP, None]

            # ---- time stepping --------------------------------------------
            for step in range(steps):
                iv, sub = step // N_SUB, step % N_SUB
                for stg in range(4):
                    sidx = 2 * sub + (0 if stg == 0 else (1 if stg < 3 else 2))
                    dxcol = (iv * NS + sidx) * INP

                    # -- small MLP: W_in then NL hidden layers (feature-major)
                    hA = None
                    hB = None
                    for layer in range(1 + NL):
                        if layer == 0:
                            wa_l = sb["wi"][:]
                            wb_l = None
                        else:
                            c0 = (layer - 1) * HH
                            wa_l = sb["wha"][:, c0 : c0 + HH]
                            wb_l = sb["whb"][:, c0 : c0 + HH]
                        pA = ps_h.tile([128, BL], f32, tag="hA")
                        pB = ps_hb.tile([HH - 128, BL], f32, tag="hB")
                        if layer == 0:
                            nc.tensor.matmul(pA[:], wa_l[:, 0:128], zT[:])
                            nc.tensor.matmul(pB[:], wa_l[:, 128:HH], zT[:])
                        else:
                            nc.tensor.matmul(
                                pA[:], wa_l[:, 0:128], hA[:], start=True, stop=False
                            )
                            nc.tensor.matmul(
                                pA[:], wb_l[:, 0:128], hB[:], start=False, stop=True
                            )
                            nc.tensor.matmul(
                                pB[:], wa_l[:, 128:HH], hA[:], start=True, stop=False
                            )
                            nc.tensor.matmul(
                                pB[:], wb_l[:, 128:HH], hB[:], start=False, stop=True
                            )
                        last = layer == NL
                        nhA = work.tile([128, BL], f16, tag="hA_sb")
                        nhB = hhB[0 : HH - 128, :] if last else work.tile(
                            [HH - 128, BL], f16, tag="hB_sb"
                        )
                        ba = sb["bias_a"][:, layer : layer + 1]
                        bb = sb["bias_b"][:, layer : layer + 1]
                        # A on ACT, B on DVE: the two bias+relu ops run
                        # concurrently instead of queueing on one engine
                        nc.scalar.activation(nhA[:], pA[:], AF.Relu, bias=ba)
                        nc.vector.tensor_scalar(nhB[:], pB[:], bb, 0.0, OP.add, OP.max)
                        hA, hB = nhA, (hhB[0 : HH - 128 + 1, :] if last else nhB)
                        pe_fill(FILL_MLP, 128)

                    # -- W_out: y[p = half*64+b, (h_local, i)]  (batch-major)
                    # region-major so each region completes early and the
                    # tanh/einsum chain starts while later regions stream
                    for rt, (h0, hc) in enumerate(REGIONS):
                        for kc in range(2):
                            lhs = hA[:] if kc == 0 else hhB[:]
                            rhs_t = sb["woa"] if kc == 0 else sb["wob"]
                            for half in range(2):
                                cols = half * HALF_COLS + h0 * INP
                                # lo/hi halves accumulate in disjoint
                                # partition rows of one bank; the sim's group
                                # guard is partition-blind, so skip it.
                                nc.tensor.matmul(
                                    yR[rt][half * 64 : half * 64 + 64, 0 : hc * INP],
                                    lhs,
                                    rhs_t[:, cols : cols + hc * INP],
                                    start=(kc == 0),
                                    stop=(kc == 1),
                                    skip_group_check=True,
                                )

                    # fillers bridge the PE gap while tanh/mult/reduce run
                    pe_fill(FILL_TAIL, 512)

                    # -- tanh -> multiply by dX -> segmented reduce over i
                    # k_t in f16: all-SBUF 2-byte operands unlock the DVE
                    # 2x/4x perf modes on the segmented reduce
                    k_t = work.tile([128, 32], f16, tag="k")
                    for rt, (h0, hc) in enumerate(REGIONS):
                        ty = ty_pool.tile([128, hc * INP], f16, tag=f"ty{rt}")
                        nc.scalar.activation(ty[:], yR[rt][:, 0 : hc * INP], AF.Tanh)
                        pr = pr_pool.tile([128, hc * INP], f16, tag=f"pr{rt}")
                        dxv = (
                            sb["dxh"][:, dxcol : dxcol + INP]
                            .unsqueeze(1)
                            .broadcast_to((128, hc, INP))
                        )
                        tyv = ty[:].rearrange("p (h i) -> p h i", i=INP)
                        prv = pr[:].rearrange("p (h i) -> p h i", i=INP)
                        nc.vector.tensor_tensor(prv, tyv, dxv, OP.mult)
                        with nc.allow_low_precision("f16 einsum accum, 42 terms"):
                            nc.vector.tensor_reduce(
                                k_t[:, h0 : h0 + hc], prv, mybir.AxisListType.X, OP.add
                            )

                    # -- next-stage input first (it gates the transpose and
                    # the whole next stage), then the zacc accumulation
                    zs16 = work.tile([128, 32], f16, tag="zs16")
                    if stg < 3:
                        nc.vector.scalar_tensor_tensor(
                            zs16[:], k_t[:], inp_w[stg], z[:], OP.mult, OP.add
                        )
                    if stg == 3:
                        nc.vector.scalar_tensor_tensor(
                            zacc[:], k_t[:], acc_w[stg], zacc[:], OP.mult, OP.add
                        )
                        nc.vector.tensor_copy(zs16[:], zacc[:])
                    ztp = ps_t.tile([HID, BL], f32, tag="ztp")
                    nc.tensor.matmul(ztp[0:32, :], zs16[:], sb["eperm"][:, 0:64])
                    nc.tensor.matmul(ztp[32:64, :], zs16[:], sb["eperm"][:, 64:128])
                    pe_fill(FILL_POST, 256)
                    # zacc off the critical path: emitted after the transpose
                    # so the DVE queue serves the next-stage input first
                    if stg == 0:
                        nc.vector.scalar_tensor_tensor(
                            zacc[:], k_t[:], acc_w[0], z[:], OP.mult, OP.add
                        )
                    elif stg < 3:
                        nc.vector.scalar_tensor_tensor(
                            zacc[:], k_t[:], acc_w[stg], zacc[:], OP.mult, OP.add
                        )
                    zT = work.tile([HID, BL], f16, tag="zT")
                    # PSUM->SBUF evacuation on ACT (idle here); frees the DVE
                    nc.scalar.activation(zT[:], ztp[:], AF.Copy)
                    if stg == 3:
                        z, zacc = zacc, z

            # ---- classifier on final state --------------------------------
            c1p = ps_h.tile([HID, BL], f32, tag="hA")
            nc.tensor.matmul(c1p[:], sb["wc1"][:], zT[:])
            c1 = work.tile([HID, BL], f16, tag="c1")
            nc.vector.tensor_scalar(c1[:], c1p[:], sb["bc1c"][:], 0.0, OP.add, OP.max)
            c2p = ps_hb.tile([NCLS, BL], f32, tag="hB")
            nc.tensor.matmul(c2p[:], sb["wc2"][:], c1[:])
            pred = work.tile([NCLS, BL], f32, tag="pred")
            nc.vector.tensor_scalar(pred[:], c2p[:], sb["bc2c"][:], None, OP.add)
            nc.sync.dma_start(out_dram[:], pred[:])

    nc.compile()
    return nc


def make_in_maps(inputs):
    shared = _prep_shared(
        inputs["W0"], inputs["b0"], inputs["W_in"], inputs["b_in"],
        inputs["W_h"], inputs["b_h"], inputs["W_out"], inputs["b_out"],
        inputs["Wc1"], inputs["bc1"], inputs["Wc2"], inputs["bc2"],
    )
    bc = np.asarray(inputs["batch_coeffs"], np.float32)
    in_maps = []
    for c in range(NCORES):
        x0t, dxh = _prep_percore(bc[c * BL : (c + 1) * BL])
        in_maps.append({**shared, "x0t": x0t, "dxh": dxh})
    return in_maps


_CACHED = {}


def kernel(**inputs):
    from concourse.bass_utils import run_bass_kernel_spmd

    if "nc" not in _CACHED:
        _CACHED["nc"] = build_nc()
    nc = _CACHED["nc"]
    in_maps = make_in_maps(inputs)
    res = run_bass_kernel_spmd(
        nc, in_maps, core_ids=list(range(NCORES)),
        trace=bool(int(os.environ.get("NCDE_TRACE", "0"))),
    )
    _CACHED["last_result"] = res
    out = np.zeros((B, NCLS), np.float32)
    for c in range(NCORES):
        out[c * BL : (c + 1) * BL, :] = res.results[c]["pred_t"].T
    return out



# revision 11
# speedup vs baseline: 1.2522x; 1.2457x over previous
"""Neural CDE forward pass on 8 Trainium2 NeuronCores (pure data parallel).

B=512 batch is sharded 64 per core.  Per core, the entire 30-step RK4
integration (120 vector-field evals) runs out of SBUF:

  - small MLP layers feature-major: lhsT = weight chunk (stationary),
    rhs = activation [feat, 64]; bias+relu on DVE for the 128-row half
    (lower access latency) and ACT for the 22-row half.
  - W_out layer batch-major, split-h: y PSUM [128 = 2 h-halves x 64 batch,
    cols = (h_local, i_pad)] so tanh/mul/reduce use all 128 lanes.
    b_out enters via a ones-row appended to the stationary activation.
  - einsum g[b,h] = sum_i tanh(y)[b,h,i] * dX[b,i]: DVE multiply with a
    broadcast dX tile (fp16) + segmented pool-avg over i (41 padded
    to 42; the /42 is folded into the scaled permutation matrices).
  - dX for the 75 distinct (interval, s) points is computed on host and
    DMA'd once.
  - z state is FEATURE-major [64h, 64b].  k_t (batch-major) is transposed
    through PE matmuls whose permutation operand is pre-scaled by the RK4
    coefficient; the zacc sum accumulates in a persistent PSUM tile across
    the 4 stages (start at k1, stop at k4), so the per-stage z bookkeeping
    is two tiny PE matmuls + one DVE add.
  - PE p-state fillers: TRN2 drops the PE clock (2.4 -> 1.2/0.65 GHz)
    after idle gaps; dummy matmuls into a scratch PSUM bank bridge the
    relu/einsum gaps so real matmuls stay at full clock.
"""

import os
import numpy as np

B, T, IN, HID, HH, NCLS = 512, 16, 41, 64, 150, 4
N_SUB, NL = 2, 3
NCORES = 8
BL = B // NCORES            # 64 per-core batch
INP = IN + 1                # 42: i padded for even segments
NIV = T - 1                 # 15 intervals
NS = 5                      # distinct s values per interval
HSTEP = 1.0 / N_SUB         # 0.5
HALF_COLS = 32 * INP        # 1344 cols per h-half
# (h0, hcount) splits of the 32 h per half; each region's cols (hcount*42)
# must fit one 2KB PSUM bank (<=512 fp32) since a matmul cannot cross banks
REGIONS = [(0, 12), (12, 12), (24, 8)]
STEPS = int(os.environ.get("NCDE_STEPS", NIV * N_SUB))  # debug knob

# filler schedules: list of rhs col-counts (graded: big early, small near
# the handoff back to real dependent matmuls)
def _fills(env, default):
    s = os.environ.get(env, default)
    return [int(x) for x in s.split(",") if x]

FILL_MLP = _fills("NCDE_FILL_MLP", "64,64,64,64,64")
FILL_TAIL = _fills("NCDE_FILL_TAIL", "512,512,512,512,512,512,512,512,512,512,256,128,128,64,64")
FILL_POST = _fills("NCDE_FILL_POST", "128,64,64")
REDUCE_KIND = os.environ.get("NCDE_REDUCE", "tr")  # tr | pool (pool: no neff)

# RK4 coefficients (per-stage input scale and accumulation weight); the
# pool-avg reduce divides by INP, so the permutation matrices also carry
# a x42 factor -- all products are exact in fp16.
ACC_W = [HSTEP / 6.0, HSTEP / 3.0, HSTEP / 3.0, HSTEP / 6.0]
INP_W = [0.5 * HSTEP, 0.5 * HSTEP, HSTEP]
RSCALE = float(INP) if REDUCE_KIND == "pool" else 1.0


def _prep_shared(W0, b0, W_in, b_in, W_h, b_h, W_out, b_out, Wc1, bc1, Wc2, bc2):
    f16 = np.float16
    f32 = np.float32
    wha = np.concatenate([W_h[i][0:128, :] for i in range(NL)], axis=1)
    whb = np.concatenate([W_h[i][128:HH, :] for i in range(NL)], axis=1)
    bias_a = np.stack([b_in[0:128]] + [b_h[i][0:128] for i in range(NL)], axis=1)
    bias_b = np.stack([b_in[128:HH]] + [b_h[i][128:HH] for i in range(NL)], axis=1)
    R = W_out.reshape(HH, HID, IN)
    Rp = np.zeros((HH, HID, INP), np.float32)
    Rp[:, :, :IN] = R
    W2 = np.concatenate(
        [Rp[:, 0:32, :].reshape(HH, HALF_COLS), Rp[:, 32:64, :].reshape(HH, HALF_COLS)],
        axis=1,
    )
    bo = np.zeros((HID, INP), np.float32)
    bo[:, :IN] = b_out.reshape(HID, IN)
    bo2 = np.concatenate([bo[0:32].reshape(-1), bo[32:64].reshape(-1)])
    i64 = np.eye(64, dtype=np.float32)
    eperm = np.vstack([np.hstack([i64, 0 * i64]), np.hstack([0 * i64, i64])])
    out = {
        "w0": W0.astype(f16),
        "b0c": b0.reshape(HID, 1).astype(f32),
        "wi": W_in.astype(f16),
        "wha": wha.astype(f16),
        "whb": whb.astype(f16),
        "bias_a": bias_a.astype(f32),
        "bias_b": bias_b.astype(f32),
        "woa": W2[0:128].astype(f16),
        "wob": np.vstack([W2[128:HH], bo2[None]]).astype(f16),
        "wc1": Wc1.astype(f16),
        "bc1c": bc1.reshape(HID, 1).astype(f32),
        "wc2": Wc2.astype(f16),
        "bc2c": bc2.reshape(NCLS, 1).astype(f32),
        "eperm": eperm.astype(f16),
        "hhB_init": np.vstack(
            [np.zeros((HH - 128, BL), np.float32), np.ones((1, BL), np.float32)]
        ).astype(f16),
    }
    # scaled permutations: transpose matmuls bake the RK4 coefficient (and
    # the x42 pool-avg compensation) into the moving operand
    scales = sorted({w * RSCALE for w in ACC_W} | {w * RSCALE for w in INP_W})
    for si, sc in enumerate(scales):
        out[f"ep{si}"] = (eperm * sc).astype(f16)
    return out, scales


def _prep_percore(bc_core):
    """bc_core: [BL, NIV, 4, IN] fp32 -> x0t [IN, BL] f16, dxh [128, NS*NIV*INP] f16."""
    x0t = bc_core[:, 0, 0, :].T.astype(np.float16)
    c1 = bc_core[:, :, 1, :]  # [BL, NIV, IN]
    c2 = bc_core[:, :, 2, :]
    c3 = bc_core[:, :, 3, :]
    dxh = np.zeros((128, NIV * NS * INP), np.float32)
    for iv in range(NIV):
        for si in range(NS):
            s = si * 0.25
            dX = c1[:, iv] + (2.0 * s) * c2[:, iv] + (3.0 * s * s) * c3[:, iv]
            col = (iv * NS + si) * INP
            dxh[0:BL, col : col + IN] = dX
            dxh[BL:128, col : col + IN] = dX
    return x0t, dxh.astype(np.float16)


def build_nc(steps=STEPS):
    """Build the single-core Bass program (same program on all 8 cores)."""
    from contextlib import ExitStack

    import concourse.bass as bass
    import concourse.mybir as mybir
    from concourse import bacc, tile

    f16 = mybir.dt.float16
    f32 = mybir.dt.float32
    AF = mybir.ActivationFunctionType
    OP = mybir.AluOpType

    nc = bacc.Bacc("TRN2", target_bir_lowering=False, debug=False)

    # host-side scale bookkeeping must match _prep_shared
    scales = sorted({w * RSCALE for w in ACC_W} | {w * RSCALE for w in INP_W})
    ep_of = {sc: f"ep{si}" for si, sc in enumerate(scales)}

    dram = {}
    ins_spec = [
        ("x0t", [IN, BL], f16),
        ("dxh", [128, NIV * NS * INP], f16),
        ("w0", [IN, HID], f16),
        ("b0c", [HID, 1], f32),
        ("wi", [HID, HH], f16),
        ("wha", [128, NL * HH], f16),
        ("whb", [HH - 128, NL * HH], f16),
        ("bias_a", [128, 1 + NL], f32),
        ("bias_b", [HH - 128, 1 + NL], f32),
        ("woa", [128, 2 * HALF_COLS], f16),
        ("wob", [HH - 128 + 1, 2 * HALF_COLS], f16),
        ("wc1", [HID, HID], f16),
        ("bc1c", [HID, 1], f32),
        ("wc2", [HID, NCLS], f16),
        ("bc2c", [NCLS, 1], f32),
        ("eperm", [128, 128], f16),
        ("hhB_init", [HH - 128 + 1, BL], f16),
    ] + [(f"ep{si}", [128, 128], f16) for si in range(len(scales))]
    for name, shape, dt in ins_spec:
        dram[name] = nc.dram_tensor(name, shape, dt, kind="ExternalInput")
    out_dram = nc.dram_tensor("pred_t", [NCLS, BL], f32, kind="ExternalOutput")

    with tile.TileContext(nc) as tc:
        with ExitStack() as ctx:
            const = ctx.enter_context(tc.tile_pool(name="const", bufs=1))
            work = ctx.enter_context(tc.tile_pool(name="work", bufs=3))
            ty_pool = ctx.enter_context(tc.tile_pool(name="ty", bufs=3))
            pr_pool = ctx.enter_context(tc.tile_pool(name="pr", bufs=3))
            ps_h = ctx.enter_context(
                tc.tile_pool(name="ps_h", bufs=1, space=bass.MemorySpace.PSUM)
            )
            ps_hb = ctx.enter_context(
                tc.tile_pool(name="ps_hb", bufs=1, space=bass.MemorySpace.PSUM)
            )
            ps_y = ctx.enter_context(
                tc.tile_pool(name="ps_y", bufs=1, space=bass.MemorySpace.PSUM)
            )
            ps_t = ctx.enter_context(
                tc.tile_pool(name="ps_t", bufs=1, space=bass.MemorySpace.PSUM)
            )
            ps_f = ctx.enter_context(
                tc.tile_pool(name="ps_f", bufs=1, space=bass.MemorySpace.PSUM)
            )

            # ---- load constants/weights into SBUF --------------------------
            sb = {}
            for name, shape, dt in ins_spec:
                t = const.tile(shape, dt, tag=name)
                nc.sync.dma_start(t[:], dram[name][:])
                sb[name] = t

            # persistent state tiles (hhB arrives with its ones row preset)
            hhB = sb["hhB_init"]
            zFa = const.tile([HID, BL], f32, tag="zFa")   # feature-major state
            zFb = const.tile([HID, BL], f32, tag="zFb")

            # psum y region tiles (persistent; serial stages reuse them)
            # one full 2KB bank each so every tile starts bank-aligned
            yR = [
                ps_y.tile([128, 512], f32, tag=f"yR{rt}", name=f"yR{rt}")
                for rt in range(len(REGIONS))
            ]

            # kFw (scaled transposed k) and zaccP (RK4 accumulator) share the
            # ps_t bank; zaccP accumulates across the 4 stages of each step
            kFw = ps_t.tile([HID, BL], f32, tag="kFw", name="kFw")
            zaccP = ps_t.tile([HID, BL], f32, tag="zaccP", name="zaccP")

            # scratch PSUM bank for p-state filler matmuls (results unread)
            fillp = ps_f.tile([128, 512], f32, tag="fillp", name="fillp")

            def pe_fill(cols_list):
                for cols in cols_list:
                    nc.tensor.matmul(
                        fillp[:, 0:cols],
                        sb["eperm"][:],
                        sb["woa"][:, 0:cols],
                        skip_group_check=True,
                    )

            # ---- initial state z0 = X0 @ W0 + b0 (feature-major) -----------
            z0p = ps_h.tile([HID, BL], f32, tag="hA")
            nc.tensor.matmul(z0p[:], sb["w0"][:], sb["x0t"][:])
            zT = work.tile([HID, BL], f16, tag="zT")
            nc.vector.tensor_scalar(zT[:], z0p[:], sb["b0c"][:], None, OP.add)
            zF = zFa
            zFn = zFb
            nc.vector.tensor_scalar(zF[:], z0p[:], sb["b0c"][:], None, OP.add)

            # ---- time stepping --------------------------------------------
            for step in range(steps):
                iv, sub = step // N_SUB, step % N_SUB
                for stg in range(4):
                    sidx = 2 * sub + (0 if stg == 0 else (1 if stg < 3 else 2))
                    dxcol = (iv * NS + sidx) * INP

                    # -- small MLP: W_in then NL hidden layers (feature-major)
                    hA = None
                    hB = None
                    for layer in range(1 + NL):
                        if layer == 0:
                            wa_l = sb["wi"][:]
                            wb_l = None
                        else:
                            c0 = (layer - 1) * HH
                            wa_l = sb["wha"][:, c0 : c0 + HH]
                            wb_l = sb["whb"][:, c0 : c0 + HH]
                        pA = ps_h.tile([128, BL], f32, tag="hA")
                        pB = ps_hb.tile([HH - 128, BL], f32, tag="hB")
                        if layer == 0:
                            nc.tensor.matmul(pA[:], wa_l[:, 0:128], zT[:])
                            nc.tensor.matmul(pB[:], wa_l[:, 128:HH], zT[:])
                        else:
                            nc.tensor.matmul(
                                pA[:], wa_l[:, 0:128], hA[:], start=True, stop=False
                            )
                            nc.tensor.matmul(
                                pA[:], wb_l[:, 0:128], hB[:], start=False, stop=True
                            )
                            nc.tensor.matmul(
                                pB[:], wa_l[:, 128:HH], hA[:], start=True, stop=False
                            )
                            nc.tensor.matmul(
                                pB[:], wb_l[:, 128:HH], hB[:], start=False, stop=True
                            )
                        last = layer == NL
                        nhA = work.tile([128, BL], f16, tag="hA_sb")
                        nhB = hhB[0 : HH - 128, :] if last else work.tile(
                            [HH - 128, BL], f16, tag="hB_sb"
                        )
                        ba = sb["bias_a"][:, layer : layer + 1]
                        bb = sb["bias_b"][:, layer : layer + 1]
                        # A (critical path) on DVE: lower access latency than
                        # ACT for small tiles; B on ACT runs concurrently
                        nc.vector.tensor_scalar(nhA[:], pA[:], ba, 0.0, OP.add, OP.max)
                        nc.scalar.activation(nhB[:], pB[:], AF.Relu, bias=bb)
                        hA, hB = nhA, (hhB[0 : HH - 128 + 1, :] if last else nhB)
                        pe_fill(FILL_MLP)

                    # -- W_out: y[p = half*64+b, (h_local, i)]  (batch-major)
                    # region-major so each region completes early and the
                    # tanh/einsum chain starts while later regions stream
                    for rt, (h0, hc) in enumerate(REGIONS):
                        for kc in range(2):
                            lhs = hA[:] if kc == 0 else hhB[:]
                            rhs_t = sb["woa"] if kc == 0 else sb["wob"]
                            for half in range(2):
                                cols = half * HALF_COLS + h0 * INP
                                # lo/hi halves accumulate in disjoint
                                # partition rows of one bank; the sim's group
                                # guard is partition-blind, so skip it.
                                nc.tensor.matmul(
                                    yR[rt][half * 64 : half * 64 + 64, 0 : hc * INP],
                                    lhs,
                                    rhs_t[:, cols : cols + hc * INP],
                                    start=(kc == 0),
                                    stop=(kc == 1),
                                    skip_group_check=True,
                                )

                    # fillers bridge the PE gap while tanh/mult/reduce run
                    pe_fill(FILL_TAIL)

                    # -- tanh -> multiply by dX -> segmented reduce over i
                    k_t = work.tile([128, 32], f16, tag="k")
                    for rt, (h0, hc) in enumerate(REGIONS):
                        ty = ty_pool.tile([128, hc * INP], f16, tag=f"ty{rt}")
                        nc.scalar.activation(ty[:], yR[rt][:, 0 : hc * INP], AF.Tanh)
                        pr = pr_pool.tile([128, hc * INP], f16, tag=f"pr{rt}")
                        dxv = (
                            sb["dxh"][:, dxcol : dxcol + INP]
                            .unsqueeze(1)
                            .broadcast_to((128, hc, INP))
                        )
                        tyv = ty[:].rearrange("p (h i) -> p h i", i=INP)
                        prv = pr[:].rearrange("p (h i) -> p h i", i=INP)
                        nc.vector.tensor_tensor(prv, tyv, dxv, OP.mult)
                        if REDUCE_KIND == "pool":
                            nc.vector.pool_avg(k_t[:, h0 : h0 + hc], prv)
                        else:
                            with nc.allow_low_precision("f16 einsum accum"):
                                nc.vector.tensor_reduce(
                                    k_t[:, h0 : h0 + hc], prv,
                                    mybir.AxisListType.X, OP.add,
                                )

                    # -- z update, feature-major: PE transposes k_t with the
                    # RK4 coefficient baked into the permutation operand
                    epa = sb[ep_of[ACC_W[stg] * RSCALE]]
                    nc.tensor.matmul(
                        zaccP[0:32, :], k_t[:], epa[:, 0:64],
                        start=(stg == 0), stop=(stg == 3), skip_group_check=True,
                    )
                    nc.tensor.matmul(
                        zaccP[32:64, :], k_t[:], epa[:, 64:128],
                        start=(stg == 0), stop=(stg == 3), skip_group_check=True,
                    )
                    zT = work.tile([HID, BL], f16, tag="zT")
                    if stg < 3:
                        epi = sb[ep_of[INP_W[stg] * RSCALE]]
                        nc.tensor.matmul(kFw[0:32, :], k_t[:], epi[:, 0:64])
                        nc.tensor.matmul(kFw[32:64, :], k_t[:], epi[:, 64:128])
                        pe_fill(FILL_POST)
                        nc.vector.tensor_tensor(zT[:], kFw[:], zF[:], OP.add)
                    else:
                        pe_fill(FILL_POST)
                        # next-step state: z + (h/6)(k1+2k2+2k3+k4)
                        nc.vector.tensor_tensor(zT[:], zaccP[:], zF[:], OP.add)
                        nc.vector.tensor_tensor(zFn[:], zaccP[:], zF[:], OP.add)
                        zF, zFn = zFn, zF

            # ---- classifier on final state --------------------------------
            c1p = ps_h.tile([HID, BL], f32, tag="hA")
            nc.tensor.matmul(c1p[:], sb["wc1"][:], zT[:])
            c1 = work.tile([HID, BL], f16, tag="c1")
            nc.vector.tensor_scalar(c1[:], c1p[:], sb["bc1c"][:], 0.0, OP.add, OP.max)
            c2p = ps_hb.tile([NCLS, BL], f32, tag="hB")
            nc.tensor.matmul(c2p[:], sb["wc2"][:], c1[:])
            pred = work.tile([NCLS, BL], f32, tag="pred")
            nc.vector.tensor_scalar(pred[:], c2p[:], sb["bc2c"][:], None, OP.add)
            nc.sync.dma_start(out_dram[:], pred[:])

    nc.compile()
    return nc


def make_in_maps(inputs):
    shared, _scales = _prep_shared(
        inputs["W0"], inputs["b0"], inputs["W_in"], inputs["b_in"],
        inputs["W_h"], inputs["b_h"], inputs["W_out"], inputs["b_out"],
        inputs["Wc1"], inputs["bc1"], inputs["Wc2"], inputs["bc2"],
    )
    bc = np.asarray(inputs["batch_coeffs"], np.float32)
    in_maps = []
    for c in range(NCORES):
        x0t, dxh = _prep_percore(bc[c * BL : (c + 1) * BL])
        in_maps.append({**shared, "x0t": x0t, "dxh": dxh})
    return in_maps


_CACHED = {}


def kernel(**inputs):
    from concourse.bass_utils import run_bass_kernel_spmd

    if "nc" not in _CACHED:
        _CACHED["nc"] = build_nc()
    nc = _CACHED["nc"]
    in_maps = make_in_maps(inputs)
    res = run_bass_kernel_spmd(
        nc, in_maps, core_ids=list(range(NCORES)),
        trace=bool(int(os.environ.get("NCDE_TRACE", "0"))),
    )
    _CACHED["last_result"] = res
    out = np.zeros((B, NCLS), np.float32)
    for c in range(NCORES):
        out[c * BL : (c + 1) * BL, :] = res.results[c]["pred_t"].T
    return out
